# revision 64
# baseline (speedup 1.0000x reference)
"""Trainium2 Bass kernel for nn_GraphPool (batched attentive FPS graph pooling).

Contract: kernel(**inputs) takes FULL inputs (B=128 graphs), shards the batch
dim across 8 NeuronCores (16 graphs each, pure data parallel), runs one SPMD
Bass program, and returns the FULL [128, 512] output.

Per-core algorithm (G=16 graphs, N=256 nodes, H=512, NH=8 heads, K=5):
  scores[g,j] = sum_{h, i<m} attn[g,h,i,j]  -> PE matmuls with block-diagonal
      0/1 mask weights (lhsT [128, 16], one nonzero column per graph) so all
      16 graphs accumulate into ONE psum tile [16, 256]; attn is DMA'd with
      row-pair interleave (i = 2p+t) giving 2KB descriptors, 2 graphs per DMA,
      masked via even/odd parity mask columns. The first attn DMA issues
      before any constant/ident work so HBM ramps immediately; the last two
      blocks are split into single-graph DMAs to shrink the exposed matmul
      work after the final attn byte lands.
  sp: load row-pair chunks, row-mask on DVE, PE-transpose into a single
      PSUM bank [128, 512], one wide ACT copy out, fused 3D colmax -> dmax;
      spT staged to DRAM for the FPS column gathers (as row gathers).
  FPS in the dmax-scaled domain: candAcc = min-chain; invalid/node-0 marks
      ride candAcc from init (via NM in minspRaw); per-iteration selected-node
      marks are added directly into candAcc during the gather window (off the
      critical path). The selected sp column is gathered with compute_op=add
      into a tile prefilled with the (static) bonus row, fusing the
      spadj = spcol + bonus add into the DMA itself.
  pool: x rows gathered with the same offsets, accumulated in-flight via
      DMA compute_op=add; LayerNorm via bn_stats/bn_aggr (eps scaled by K^2).

All mask/iota constants are precomputed on the host and passed as extra
inputs — deriving them on-chip serialized the first ~35us of the kernel.
(tensor_tensor_reduce and indirect compute_op min/max are avoided: rejected
or exec-unit-wedging on this HW; compute_op=add is fine.)
"""

import os
import sys
from contextlib import ExitStack

for _p in ("/opt/trn_rl_repo", "/root/.axon_site/_ro/trn_rl_repo"):
    if os.path.isdir(_p) and _p not in sys.path:
        sys.path.append(_p)

import numpy as np

import concourse.mybir as mybir
from concourse.bass import Bass, IndirectOffsetOnAxis
from concourse.bacc import Bacc
from concourse.masks import make_identity
from concourse.tile import TileContext

B, N, H, NH, K = 128, 256, 512, 8, 5
NCORES = 8
G = B // NCORES  # graphs per core
P = 128
LN_EPS = 1e-5
BIG = 1.0e30  # unavailable-node mark

f32 = mybir.dt.float32
f32r = mybir.dt.float32r
i32 = mybir.dt.int32
u32 = mybir.dt.uint32
AX = mybir.AxisListType
OP = mybir.AluOpType

TRACE = False
LAST_RESULT = None
GPER = 2  # graphs per attn DMA (full blocks)
NSPLIT = 4  # trailing graphs streamed as single-graph DMAs


def build_bass(trivial_affine: bool = False) -> Bass:
    nc = Bacc()
    x = nc.dram_tensor("x", [G, N, H], f32, kind="ExternalInput")
    attn = nc.dram_tensor("attn", [G, NH, N, N], f32, kind="ExternalInput")
    sp = nc.dram_tensor("spatial_pos", [G, N, N], f32, kind="ExternalInput")
    xm = nc.dram_tensor("x_mask", [G, N], f32, kind="ExternalInput")
    # host-precomputed constants (see core_inputs)
    xbde_d = nc.dram_tensor("c_xbde", [P, G * G], f32r, kind="ExternalInput")
    xbdo_d = nc.dram_tensor("c_xbdo", [P, G * G], f32r, kind="ExternalInput")
    xmt2_d = nc.dram_tensor("c_xmt2", [P, 2 * G], f32, kind="ExternalInput")
    nmpre_d = nc.dram_tensor("c_nmpre", [G, N], f32, kind="ExternalInput")
    iotaf_d = nc.dram_tensor("c_iotaf", [G, N], f32, kind="ExternalInput")
    rowbi_d = nc.dram_tensor("c_rowbi", [G, 1], i32, kind="ExternalInput")
    rowbf_d = nc.dram_tensor("c_rowbf", [G, 1], f32, kind="ExternalInput")
    if not trivial_affine:
        gb_d = nc.dram_tensor("c_gb", [G, H], f32, kind="ExternalInput")
        bb_d = nc.dram_tensor("c_bb", [G, H], f32, kind="ExternalInput")
    out = nc.dram_tensor("out", [G, H], f32, kind="ExternalOutput")
    spt_dram = nc.dram_tensor("spt_scratch", [G, N, N], f32, kind="Internal")

    x_flat = x[:].rearrange("g n h -> (g n) h")
    spt_flat = spt_dram[:].rearrange("g n j -> (g n) j")

    with TileContext(nc) as tc, ExitStack() as ctx:
        cpool = ctx.enter_context(tc.tile_pool(name="cpool", bufs=1))
        small = ctx.enter_context(tc.tile_pool(name="small", bufs=2))
        fps = ctx.enter_context(tc.tile_pool(name="fps", bufs=2))
        attn_pool = ctx.enter_context(tc.tile_pool(name="attn_pool", bufs=6))
        sp_pool = ctx.enter_context(tc.tile_pool(name="sp_pool", bufs=6))
        spt_pool = ctx.enter_context(tc.tile_pool(name="spt_pool", bufs=4))
        psum_sc = ctx.enter_context(tc.tile_pool(name="psum_sc", bufs=1, space="PSUM"))
        psum_tr = ctx.enter_context(tc.tile_pool(name="psum_tr", bufs=4, space="PSUM"))
        psum_mi = ctx.enter_context(tc.tile_pool(name="psum_mi", bufs=1, space="PSUM"))

        # ---- streaming DMA issuers (data first, constants after) ----
        # Single-graph attn pieces (2MB each; 16KB/partition tiles) with the
        # final graph tapered into head slices so the last byte to land gates
        # only a handful of matmuls. Pieces alternate between the sync and
        # scalar HWDGE queues (the only two): a single queue hands off
        # back-to-back blocks with a ~1.5us HBM gap per boundary and
        # serializes the drain.
        # g15 is NOT in the regular piece list: its 2MB loads early via the
        # gpsimd SWDGE queue into a resident tile, and its matmuls are
        # emitted before g14's taper — so the last byte to land is g14's
        # final 512KB piece, gating only 4 matmuls. Pieces alternate between
        # the sync and scalar HWDGE queues.
        PIECES = [(g, 0, NH) for g in range(G - 2)]
        PIECES += [(G - 2, 0, NH // 2), (G - 2, NH // 2, NH // 4),
                   (G - 2, 3 * NH // 4, NH // 4)]
        attn_qs = (nc.sync, nc.scalar)
        attn_qi = [0]

        def attn_dma(piece):
            g0, h0, hcnt = piece
            at = attn_pool.tile([P, 1, hcnt, 2, N], f32r, tag="at")
            eng = attn_qs[attn_qi[0] % len(attn_qs)]
            attn_qi[0] += 1
            eng.dma_start(
                at,
                attn[g0 : g0 + 1, h0 : h0 + hcnt]
                .rearrange("g h (p t) j -> p g h t j", t=2)
                .bitcast(f32r),
            )
            return at

        # sp loads: per-graph 256KB DMAs on scalar, ALL front-loaded into the
        # first few pieces (16-buf pool, no recycling waits) so every sp
        # block lands by ~40us — a late sp block delays CMall -> FPS prep
        # past the end of the stream. Partition p holds rows p and 128+p.
        sp_tiles = {}

        def sp_dma(g):
            spin = sp_pool.tile([P, 2, N], f32, tag="spin")
            nc.scalar.dma_start(spin, sp[g].rearrange("(c p) j -> p c j", c=2))
            sp_tiles[g] = spin

        # first attn pieces on BOTH queues + sp0 go out before anything
        # else so HBM ramps immediately
        at0 = attn_dma(PIECES[0])
        at_next = attn_dma(PIECES[1])
        xmT2 = cpool.tile([P, 2 * G], f32)
        nc.scalar.dma_start(xmT2, xmt2_d[:, :])
        sp_dma(0)

        # ---- constants ----
        XBDe = cpool.tile([P, G * G], f32r)
        nc.sync.dma_start(XBDe, xbde_d[:, :])
        XBDo = cpool.tile([P, G * G], f32r)
        nc.sync.dma_start(XBDo, xbdo_d[:, :])
        XBD = (XBDe, XBDo)

        ident = cpool.tile([P, P], f32)
        make_identity(nc, ident)

        CMall = cpool.tile([P, 2 * G], f32)
        scores_ps = psum_sc.tile([G, N], f32)

        # ---- streaming compute blocks ----
        def sp_proc(g):
            spin = sp_tiles.pop(g)
            # row masking (invalid node rows -> 0) on gpsimd
            nc.gpsimd.tensor_mul(
                spin[:, 0, :],
                spin[:, 0, :],
                xmT2[:, 2 * g : 2 * g + 1].to_broadcast([P, N]),
            )
            nc.gpsimd.tensor_mul(
                spin[:, 1, :],
                spin[:, 1, :],
                xmT2[:, 2 * g + 1 : 2 * g + 2].to_broadcast([P, N]),
            )
            # 4 PE transposes into one PSUM bank: [:, jc, :] = spT chunk jc
            pt = psum_tr.tile([P, 2, N], f32, tag="ptr")
            for jc in range(2):
                for ic in range(2):
                    nc.tensor.transpose(
                        pt[:, jc, ic * P : (ic + 1) * P],
                        spin[:, ic, jc * P : (jc + 1) * P],
                        ident,
                    )
            # one wide PSUM->SBUF copy on ACT
            sptw = spt_pool.tile([P, 2, N], f32, tag="sptw")
            nc.scalar.copy(sptw, pt[:, :, :])
            # fused colmax over both chunks -> CMall[:, 2g:2g+2], then mask
            cmv = CMall[:].rearrange("p (h c) -> p h c", c=2)[:, g, :]
            nc.vector.reduce_max(cmv, sptw, axis=AX.X)
            nc.vector.tensor_mul(cmv, cmv, xmT2[:, 2 * g : 2 * g + 2])
            # stage spT to DRAM for indirect row gathers; gpsimd SWDGE keeps
            # this write traffic off the scalar HWDGE queue (which now also
            # carries alternate attn blocks)
            nc.gpsimd.dma_start(
                spt_dram[g].rearrange("(c p) i -> p c i", c=2), sptw
            )

        def attn_mm(piece, at, is_first, is_last):
            g, _h0, hcnt = piece[0], piece[1], piece[2]
            for t in range(2):
                for h in range(hcnt):
                    idx = t * hcnt + h
                    nc.tensor.matmul(
                        scores_ps,
                        XBD[t][:, g * G : (g + 1) * G],
                        at[:, 0, h, t, :],
                        start=(is_first and idx == 0),
                        stop=(is_last and idx == 2 * hcnt - 1),
                    )

        # interleave: sp (and FPS prep below) completes ~2/3 through attn
        rowbase_i = cpool.tile([G, 1], i32)
        rowbase_f = cpool.tile([G, 1], f32)
        NMpre = cpool.tile([G, N], f32)
        XM = cpool.tile([G, N], f32)
        iota_f = cpool.tile([G, N], f32)
        Mtile = cpool.tile([P, G], f32)
        MT = small.tile([G, P], f32)
        dmax = cpool.tile([G, 1], f32)
        minspRaw = cpool.tile([G, N], f32)
        xsum = cpool.tile([G, H], f32)
        at15 = cpool.tile([P, 1, NH, 2, N], f32r)
        if not trivial_affine:
            gb = cpool.tile([G, H], f32)
            bb = cpool.tile([G, H], f32)

        # sp issue/process interleave: 2 per piece early, 1 per piece after
        SP_DMAS = {0: (1,), 1: (2, 3), 2: (4, 5), 3: (6, 7)}
        SP_DMAS.update({q: (4 + q,) for q in range(4, 12)})
        SP_PROCS = {0: (0, 1), 1: (2, 3), 2: (4, 5), 3: (6, 7)}
        SP_PROCS.update({q: (4 + q,) for q in range(4, 12)})

        for q, piece in enumerate(PIECES):
            at_cur = at0 if q == 0 else at_next
            for gi in SP_DMAS.get(q, ()):
                sp_dma(gi)
            if 0 < q + 1 < len(PIECES):
                at_next = attn_dma(PIECES[q + 1])
            for gi in SP_PROCS.get(q, ()):
                sp_proc(gi)
            if q == len(PIECES) - 5:
                # g15's data has been resident since ~100us; emitting its
                # matmuls here lets PE chew them before the last pieces
                # land, shrinking the post-stream matmul backlog
                attn_mm((G - 1, 0, NH), at15, is_first=False, is_last=False)
            attn_mm(
                piece, at_cur,
                is_first=(q == 0), is_last=(q == len(PIECES) - 1),
            )
            if q == 4:
                # needed by the FPS-prep gathers mid-stream
                nc.scalar.dma_start(rowbase_i, rowbi_d[:, :])
                nc.scalar.dma_start(rowbase_f, rowbf_d[:, :])
                nc.scalar.dma_start(NMpre, nmpre_d[:, :])
            if q == 11:
                # tail-only constants (gpsimd SWDGE keeps the HWDGE queues
                # clear for attn), then FPS prep, then g15's early attn load
                nc.gpsimd.dma_start(XM, xm[:, :])
                nc.gpsimd.dma_start(iota_f, iotaf_d[:, :])
                if not trivial_affine:
                    nc.gpsimd.dma_start(gb, gb_d[:, :])
                    nc.gpsimd.dma_start(bb, bb_d[:, :])
                # ---- FPS prep (depends only on sp) ----
                nc.vector.reduce_max(
                    Mtile, CMall[:].rearrange("p (h c) -> p h c", c=2), axis=AX.X
                )
                pmt = psum_mi.tile([G, P], f32, tag="pmt")
                nc.tensor.transpose(pmt, Mtile, ident)
                nc.vector.tensor_copy(MT, pmt)
                nc.vector.reduce_max(dmax, MT, axis=AX.X)
                nc.gpsimd.indirect_dma_start(
                    out=minspRaw,
                    out_offset=None,
                    in_=spt_flat,
                    in_offset=IndirectOffsetOnAxis(ap=rowbase_i[:, :1], axis=0),
                )
                nc.vector.tensor_add(minspRaw, minspRaw, NMpre)
                nc.gpsimd.indirect_dma_start(
                    out=xsum,
                    out_offset=None,
                    in_=x_flat,
                    in_offset=IndirectOffsetOnAxis(ap=rowbase_i[:, :1], axis=0),
                )
            if q == 8:
                # g15's early attn load (gpsimd SWDGE; resident by ~100us so
                # its matmuls can run in PE's idle window at piece 12)
                nc.gpsimd.dma_start(
                    at15,
                    attn[G - 1 : G]
                    .rearrange("g h (p t) j -> p g h t j", t=2)
                    .bitcast(f32r),
                )

        # ---- tail: scores -> bonus -> FPS iterations ----
        # masked scores straight out of PSUM (fused copy+mask), then smax
        scoresAll = cpool.tile([G, N], f32)
        nc.vector.tensor_mul(scoresAll, scores_ps, XM)
        smax = small.tile([G, 1], f32)
        nc.vector.reduce_max(smax, scoresAll, axis=AX.X)
        inv_smax = small.tile([G, 1], f32)
        nc.vector.reciprocal(inv_smax, smax)
        # bonusM = scores * (0.1 * dmax / smax) + NM  (dmax-scaled domain;
        # carries the -BIG marks of invalid + already-selected nodes)
        sfac = small.tile([G, 1], f32)
        nc.vector.tensor_scalar(
            sfac, inv_smax, dmax[:, :1], 0.1, op0=OP.mult, op1=OP.mult
        )
        bonusM = cpool.tile([G, N], f32)
        nc.vector.tensor_scalar(bonusM, scoresAll, sfac[:, :1], None, op0=OP.mult)

        # cand = candAcc (min-chain with bonus folded in; marks live in bonusM
        # and propagate through the min since spcol+bonusM >= -BIG there)
        candAcc = cpool.tile([G, N], f32)
        nc.vector.tensor_add(candAcc, minspRaw, bonusM)
        for t in range(1, K):
            mx8 = small.tile([G, 8], f32, tag="mx8")
            nc.vector.max(out=mx8, in_=candAcc)
            ix8 = small.tile([G, 8], u32, tag="ix8")
            nc.vector.max_index(ix8, mx8, candAcc)
            offi = small.tile([G, 1], i32, tag="offi")
            nc.vector.tensor_scalar(
                offi, ix8[:, 0:1], rowbase_f[:, :1], None, op0=OP.add
            )
            if t < K - 1:
                spcol = fps.tile([G, N], f32, tag="spcol")
                nc.gpsimd.indirect_dma_start(
                    out=spcol,
                    out_offset=None,
                    in_=spt_flat,
                    in_offset=IndirectOffsetOnAxis(ap=offi[:, :1], axis=0),
                )
            # accumulate the selected x row in-flight; the last row goes to
            # its own tile via a plain gather (a cce-add would serialize
            # behind the previous cce-add's completion, delaying the LN)
            if t < K - 1:
                nc.gpsimd.indirect_dma_start(
                    out=xsum,
                    out_offset=None,
                    in_=x_flat,
                    in_offset=IndirectOffsetOnAxis(ap=offi[:, :1], axis=0),
                    compute_op=OP.add,
                )
            else:
                x5 = cpool.tile([G, H], f32)
                nc.gpsimd.indirect_dma_start(
                    out=x5,
                    out_offset=None,
                    in_=x_flat,
                    in_offset=IndirectOffsetOnAxis(ap=offi[:, :1], axis=0),
                )
            if t < K - 1:
                # mark the selected node in bonusM (overlaps the gather)
                idxf = small.tile([G, 1], f32, tag="idxf")
                nc.vector.tensor_copy(idxf, ix8[:, 0:1])
                ohB = fps.tile([G, N], f32, tag="ohB")
                nc.vector.tensor_scalar(
                    ohB, iota_f, idxf[:, :1], -BIG, op0=OP.is_equal, op1=OP.mult
                )
                nc.vector.tensor_add(bonusM, bonusM, ohB)
                spadj = fps.tile([G, N], f32, tag="spadj")
                nc.vector.tensor_add(spadj, spcol, bonusM)
                nc.vector.tensor_tensor(candAcc, candAcc, spadj, op=OP.min)

        # ---- LayerNorm on xsum (mean of K rows; eps scaled by K^2) ----
        nc.vector.tensor_add(xsum, xsum, x5)
        st6 = small.tile([G, 6], f32)
        nc.vector.bn_stats(st6, xsum)
        mv = small.tile([G, 2], f32)
        nc.vector.bn_aggr(mv, st6)
        veps = small.tile([G, 1], f32)
        nc.vector.tensor_scalar(veps, mv[:, 1:2], float(K * K) * LN_EPS, None, op0=OP.add)
        std = small.tile([G, 1], f32)
        nc.scalar.sqrt(std, veps)
        rstd = small.tile([G, 1], f32)
        nc.vector.reciprocal(rstd, std)
        xn = cpool.tile([G, H], f32)
        nc.vector.tensor_scalar(
            xn, xsum, mv[:, 0:1], rstd[:, :1], op0=OP.subtract, op1=OP.mult
        )
        if trivial_affine:
            nc.sync.dma_start(out[:, :], xn)
        else:
            outt = cpool.tile([G, H], f32)
            nc.vector.tensor_mul(outt, xn, gb)
            nc.vector.tensor_add(outt, outt, bb)
            nc.sync.dma_start(out[:, :], outt)

    nc.compile()
    return nc


def core_inputs(core: int, x, attn, sp, xm, gamma, beta, trivial_affine=False) -> dict:
    """Per-core input map incl. host-precomputed constants."""
    sl = slice(core * G, (core + 1) * G)
    xmc = np.ascontiguousarray(xm[sl])  # [G, N]
    lens = xmc.sum(axis=1).astype(np.int32)  # [G]

    pidx = np.arange(P)
    # chunk masks: node p (c=0) / node 128+p (c=1) valid, interleaved pairs
    xmt2 = np.zeros((P, 2 * G), dtype=np.float32)
    xmt2[:, 0::2] = (pidx[:, None] < lens[None, :]).astype(np.float32)
    xmt2[:, 1::2] = ((pidx[:, None] + P) < lens[None, :]).astype(np.float32)
    # parity masks: node 2p / 2p+1 valid, in block-diagonal layout
    xme = (2 * pidx[:, None] < lens[None, :]).astype(np.float32)
    xmo = ((2 * pidx[:, None] + 1) < lens[None, :]).astype(np.float32)
    xbde = np.zeros((P, G * G), dtype=np.float32)
    xbdo = np.zeros((P, G * G), dtype=np.float32)
    for g in range(G):
        xbde[:, g * G + g] = xme[:, g]
        xbdo[:, g * G + g] = xmo[:, g]
    # NM: 0 on available nodes, -BIG on invalid + node 0
    nmpre = (xmc - 1.0) * BIG
    nmpre[:, 0] = -BIG
    iotaf = np.broadcast_to(
        np.arange(N, dtype=np.float32)[None, :], (G, N)
    ).copy()
    rowb = (np.arange(G, dtype=np.int64) * N).reshape(G, 1)
    ret_affine = {}
    if not trivial_affine:
        ret_affine = {
            "c_gb": np.broadcast_to(gamma.reshape(1, H), (G, H)).copy().astype(np.float32),
            "c_bb": np.broadcast_to(beta.reshape(1, H), (G, H)).copy().astype(np.float32),
        }
    return {
        **ret_affine,
        "x": np.ascontiguousarray(x[sl]),
        "attn": np.ascontiguousarray(attn[sl]),
        "spatial_pos": np.ascontiguousarray(sp[sl]),
        "x_mask": xmc,
        "c_xbde": xbde,
        "c_xbdo": xbdo,
        "c_xmt2": xmt2,
        "c_nmpre": nmpre.astype(np.float32),
        "c_iotaf": iotaf,
        "c_rowbi": rowb.astype(np.int32),
        "c_rowbf": rowb.astype(np.float32),
    }


_NC_CACHE = None


def kernel(**inputs) -> np.ndarray:
    global _NC_CACHE, LAST_RESULT
    from concourse.bass_utils import run_bass_kernel_spmd

    x = np.ascontiguousarray(np.asarray(inputs["x"]), dtype=np.float32)
    attn = np.ascontiguousarray(np.asarray(inputs["attn"]), dtype=np.float32)
    sp = np.ascontiguousarray(np.asarray(inputs["spatial_pos"]), dtype=np.float32)
    xm = np.ascontiguousarray(np.asarray(inputs["x_mask"]), dtype=np.float32)
    gamma = np.asarray(inputs["gamma"], dtype=np.float32)
    beta = np.asarray(inputs["beta"], dtype=np.float32)

    trivial = bool(np.all(gamma == 1.0) and np.all(beta == 0.0))
    if _NC_CACHE is None or _NC_CACHE[0] != trivial:
        _NC_CACHE = (trivial, build_bass(trivial_affine=trivial))
    nc = _NC_CACHE[1]

    in_maps = [
        core_inputs(c, x, attn, sp, xm, gamma, beta, trivial_affine=trivial)
        for c in range(NCORES)
    ]

    res = run_bass_kernel_spmd(
        nc, in_maps, core_ids=list(range(NCORES)), trace=TRACE
    )
    LAST_RESULT = res
    return np.concatenate([r["out"] for r in res.results], axis=0)


# revision 65
# speedup vs baseline: 1.1027x; 1.1027x over previous
"""Trainium2 Bass kernel for nn_GraphPool (batched attentive FPS graph pooling).

Contract: kernel(**inputs) takes FULL inputs (B=128 graphs), shards the batch
dim across 8 NeuronCores (16 graphs each, pure data parallel), runs one SPMD
Bass program, and returns the FULL [128, 512] output.

Per-core algorithm (G=16 graphs, N=256 nodes, H=512, NH=8 heads, K=5):
  scores[g,j] = sum_{h, i<m} attn[g,h,i,j]  -> PE matmuls with block-diagonal
      0/1 mask weights (lhsT [128, 16], one nonzero column per graph) so all
      16 graphs accumulate into ONE psum tile [16, 256]; attn is DMA'd with
      row-pair interleave (i = 2p+t) giving 2KB descriptors, 2 graphs per DMA,
      masked via even/odd parity mask columns. The first attn DMA issues
      before any constant/ident work so HBM ramps immediately; the last two
      blocks are split into single-graph DMAs to shrink the exposed matmul
      work after the final attn byte lands.
  sp: load row-pair chunks, row-mask on DVE, PE-transpose into a single
      PSUM bank [128, 512], one wide ACT copy out, fused 3D colmax -> dmax;
      spT staged to DRAM for the FPS column gathers (as row gathers).
  FPS in the dmax-scaled domain: candAcc = min-chain; invalid/node-0 marks
      ride candAcc from init (via NM in minspRaw); per-iteration selected-node
      marks are added directly into candAcc during the gather window (off the
      critical path). The selected sp column is gathered with compute_op=add
      into a tile prefilled with the (static) bonus row, fusing the
      spadj = spcol + bonus add into the DMA itself.
  pool: x rows gathered with the same offsets, accumulated in-flight via
      DMA compute_op=add; LayerNorm via bn_stats/bn_aggr (eps scaled by K^2).

All mask/iota constants are precomputed on the host and passed as extra
inputs — deriving them on-chip serialized the first ~35us of the kernel.
(tensor_tensor_reduce and indirect compute_op min/max are avoided: rejected
or exec-unit-wedging on this HW; compute_op=add is fine.)
"""

import os
import sys
from contextlib import ExitStack

for _p in ("/opt/trn_rl_repo", "/root/.axon_site/_ro/trn_rl_repo"):
    if os.path.isdir(_p) and _p not in sys.path:
        sys.path.append(_p)

import numpy as np

import concourse.mybir as mybir
from concourse.bass import Bass, IndirectOffsetOnAxis
from concourse.bacc import Bacc
from concourse.masks import make_identity
from concourse.tile import TileContext

B, N, H, NH, K = 128, 256, 512, 8, 5
NCORES = 8
G = B // NCORES  # graphs per core
P = 128
LN_EPS = 1e-5
BIG = 1.0e30  # unavailable-node mark

f32 = mybir.dt.float32
f32r = mybir.dt.float32r
i32 = mybir.dt.int32
u32 = mybir.dt.uint32
AX = mybir.AxisListType
OP = mybir.AluOpType

TRACE = False
LAST_RESULT = None
GPER = 2  # graphs per attn DMA (full blocks)
NSPLIT = 4  # trailing graphs streamed as single-graph DMAs


def build_bass(trivial_affine: bool = False) -> Bass:
    nc = Bacc()
    x = nc.dram_tensor("x", [G, N, H], f32, kind="ExternalInput")
    attn = nc.dram_tensor("attn", [G, NH, N, N], f32, kind="ExternalInput")
    sp = nc.dram_tensor("spatial_pos", [G, N, N], f32, kind="ExternalInput")
    xm = nc.dram_tensor("x_mask", [G, N], f32, kind="ExternalInput")
    # host-precomputed constants (see core_inputs)
    xbde_d = nc.dram_tensor("c_xbde", [P, G * G], f32r, kind="ExternalInput")
    xbdo_d = nc.dram_tensor("c_xbdo", [P, G * G], f32r, kind="ExternalInput")
    xmt2_d = nc.dram_tensor("c_xmt2", [P, 2 * G], f32, kind="ExternalInput")
    nmpre_d = nc.dram_tensor("c_nmpre", [G, N], f32, kind="ExternalInput")
    iotaf_d = nc.dram_tensor("c_iotaf", [G, N], f32, kind="ExternalInput")
    rowbi_d = nc.dram_tensor("c_rowbi", [G, 1], i32, kind="ExternalInput")
    rowbf_d = nc.dram_tensor("c_rowbf", [G, 1], f32, kind="ExternalInput")
    if not trivial_affine:
        gb_d = nc.dram_tensor("c_gb", [G, H], f32, kind="ExternalInput")
        bb_d = nc.dram_tensor("c_bb", [G, H], f32, kind="ExternalInput")
    out = nc.dram_tensor("out", [G, H], f32, kind="ExternalOutput")
    spt_dram = nc.dram_tensor("spt_scratch", [G, N, N], f32, kind="Internal")

    x_flat = x[:].rearrange("g n h -> (g n) h")
    spt_flat = spt_dram[:].rearrange("g n j -> (g n) j")

    with TileContext(nc) as tc, ExitStack() as ctx:
        cpool = ctx.enter_context(tc.tile_pool(name="cpool", bufs=1))
        small = ctx.enter_context(tc.tile_pool(name="small", bufs=2))
        fps = ctx.enter_context(tc.tile_pool(name="fps", bufs=2))
        attn_pool = ctx.enter_context(tc.tile_pool(name="attn_pool", bufs=6))
        sp_pool = ctx.enter_context(tc.tile_pool(name="sp_pool", bufs=6))
        spt_pool = ctx.enter_context(tc.tile_pool(name="spt_pool", bufs=4))
        psum_sc = ctx.enter_context(tc.tile_pool(name="psum_sc", bufs=1, space="PSUM"))
        psum_tr = ctx.enter_context(tc.tile_pool(name="psum_tr", bufs=4, space="PSUM"))
        psum_mi = ctx.enter_context(tc.tile_pool(name="psum_mi", bufs=1, space="PSUM"))

        # ---- streaming DMA issuers (data first, constants after) ----
        # Single-graph attn pieces (2MB each; 16KB/partition tiles) with the
        # final graph tapered into head slices so the last byte to land gates
        # only a handful of matmuls. Pieces alternate between the sync and
        # scalar HWDGE queues (the only two): a single queue hands off
        # back-to-back blocks with a ~1.5us HBM gap per boundary and
        # serializes the drain.
        # g15 is NOT in the regular piece list: its 2MB loads early via the
        # gpsimd SWDGE queue into a resident tile, and its matmuls are
        # emitted before g14's taper — so the last byte to land is g14's
        # final 512KB piece, gating only 4 matmuls. Pieces alternate between
        # the sync and scalar HWDGE queues.
        PIECES = [(g, 0, NH) for g in range(G - 2)]
        PIECES += [(G - 2, 0, NH // 2), (G - 2, NH // 2, NH // 4),
                   (G - 2, 3 * NH // 4, NH // 4)]
        attn_qs = (nc.sync, nc.scalar)
        attn_qi = [0]

        def attn_dma(piece):
            g0, h0, hcnt = piece
            at = attn_pool.tile([P, 1, hcnt, 2, N], f32r, tag="at")
            eng = attn_qs[attn_qi[0] % len(attn_qs)]
            attn_qi[0] += 1
            eng.dma_start(
                at,
                attn[g0 : g0 + 1, h0 : h0 + hcnt]
                .rearrange("g h (p t) j -> p g h t j", t=2)
                .bitcast(f32r),
            )
            return at

        # sp loads: per-graph 256KB DMAs on scalar, ALL front-loaded into the
        # first few pieces (16-buf pool, no recycling waits) so every sp
        # block lands by ~40us — a late sp block delays CMall -> FPS prep
        # past the end of the stream. Partition p holds rows p and 128+p.
        sp_tiles = {}

        def sp_dma(g):
            spin = sp_pool.tile([P, 2, N], f32, tag="spin")
            nc.scalar.dma_start(spin, sp[g].rearrange("(c p) j -> p c j", c=2))
            sp_tiles[g] = spin

        # first attn + sp data DMAs go out before anything else
        at0 = attn_dma(PIECES[0])
        xmT2 = cpool.tile([P, 2 * G], f32)
        nc.scalar.dma_start(xmT2, xmt2_d[:, :])
        sp_dma(0)

        # ---- constants ----
        XBDe = cpool.tile([P, G * G], f32r)
        nc.sync.dma_start(XBDe, xbde_d[:, :])
        XBDo = cpool.tile([P, G * G], f32r)
        nc.sync.dma_start(XBDo, xbdo_d[:, :])
        XBD = (XBDe, XBDo)

        ident = cpool.tile([P, P], f32)
        make_identity(nc, ident)

        CMall = cpool.tile([P, 2 * G], f32)
        scores_ps = psum_sc.tile([G, N], f32)

        # ---- streaming compute blocks ----
        def sp_proc(g):
            spin = sp_tiles.pop(g)
            # row masking (invalid node rows -> 0) on gpsimd
            nc.gpsimd.tensor_mul(
                spin[:, 0, :],
                spin[:, 0, :],
                xmT2[:, 2 * g : 2 * g + 1].to_broadcast([P, N]),
            )
            nc.gpsimd.tensor_mul(
                spin[:, 1, :],
                spin[:, 1, :],
                xmT2[:, 2 * g + 1 : 2 * g + 2].to_broadcast([P, N]),
            )
            # 4 PE transposes into one PSUM bank: [:, jc, :] = spT chunk jc
            pt = psum_tr.tile([P, 2, N], f32, tag="ptr")
            for jc in range(2):
                for ic in range(2):
                    nc.tensor.transpose(
                        pt[:, jc, ic * P : (ic + 1) * P],
                        spin[:, ic, jc * P : (jc + 1) * P],
                        ident,
                    )
            # one wide PSUM->SBUF copy on ACT
            sptw = spt_pool.tile([P, 2, N], f32, tag="sptw")
            nc.scalar.copy(sptw, pt[:, :, :])
            # fused colmax over both chunks -> CMall[:, 2g:2g+2], then mask
            cmv = CMall[:].rearrange("p (h c) -> p h c", c=2)[:, g, :]
            nc.vector.reduce_max(cmv, sptw, axis=AX.X)
            nc.vector.tensor_mul(cmv, cmv, xmT2[:, 2 * g : 2 * g + 2])
            # stage spT to DRAM for indirect row gathers; gpsimd SWDGE keeps
            # this write traffic off the scalar HWDGE queue (which now also
            # carries alternate attn blocks)
            nc.gpsimd.dma_start(
                spt_dram[g].rearrange("(c p) i -> p c i", c=2), sptw
            )

        def attn_mm(piece, at, is_first, is_last):
            g, _h0, hcnt = piece[0], piece[1], piece[2]
            for t in range(2):
                for h in range(hcnt):
                    idx = t * hcnt + h
                    nc.tensor.matmul(
                        scores_ps,
                        XBD[t][:, g * G : (g + 1) * G],
                        at[:, 0, h, t, :],
                        start=(is_first and idx == 0),
                        stop=(is_last and idx == 2 * hcnt - 1),
                    )

        # interleave: sp (and FPS prep below) completes ~2/3 through attn
        rowbase_i = cpool.tile([G, 1], i32)
        rowbase_f = cpool.tile([G, 1], f32)
        NMpre = cpool.tile([G, N], f32)
        XM = cpool.tile([G, N], f32)
        iota_f = cpool.tile([G, N], f32)
        Mtile = cpool.tile([P, G], f32)
        MT = small.tile([G, P], f32)
        dmax = cpool.tile([G, 1], f32)
        minspRaw = cpool.tile([G, N], f32)
        xsum = cpool.tile([G, H], f32)
        at15 = cpool.tile([P, 1, NH, 2, N], f32r)
        if not trivial_affine:
            gb = cpool.tile([G, H], f32)
            bb = cpool.tile([G, H], f32)

        # sp issue/process interleave: 2 per piece early, 1 per piece after
        SP_DMAS = {0: (1,), 1: (2, 3), 2: (4, 5), 3: (6, 7)}
        SP_DMAS.update({q: (4 + q,) for q in range(4, 12)})
        SP_PROCS = {0: (0, 1), 1: (2, 3), 2: (4, 5), 3: (6, 7)}
        SP_PROCS.update({q: (4 + q,) for q in range(4, 12)})

        at_next = None
        for q, piece in enumerate(PIECES):
            at_cur = at0 if q == 0 else at_next
            for gi in SP_DMAS.get(q, ()):
                sp_dma(gi)
            if q + 1 < len(PIECES):
                at_next = attn_dma(PIECES[q + 1])
            for gi in SP_PROCS.get(q, ()):
                sp_proc(gi)
            if q == len(PIECES) - 3:
                # g15's data has been resident since mid-stream; run its
                # matmuls ahead of g14's taper
                attn_mm((G - 1, 0, NH), at15, is_first=False, is_last=False)
            attn_mm(
                piece, at_cur,
                is_first=(q == 0), is_last=(q == len(PIECES) - 1),
            )
            if q == 4:
                # needed by the FPS-prep gathers mid-stream
                nc.scalar.dma_start(rowbase_i, rowbi_d[:, :])
                nc.scalar.dma_start(rowbase_f, rowbf_d[:, :])
                nc.scalar.dma_start(NMpre, nmpre_d[:, :])
            if q == 11:
                # tail-only constants (gpsimd SWDGE keeps the HWDGE queues
                # clear for attn), then FPS prep, then g15's early attn load
                nc.gpsimd.dma_start(XM, xm[:, :])
                nc.gpsimd.dma_start(iota_f, iotaf_d[:, :])
                if not trivial_affine:
                    nc.gpsimd.dma_start(gb, gb_d[:, :])
                    nc.gpsimd.dma_start(bb, bb_d[:, :])
                # ---- FPS prep (depends only on sp) ----
                nc.vector.reduce_max(
                    Mtile, CMall[:].rearrange("p (h c) -> p h c", c=2), axis=AX.X
                )
                pmt = psum_mi.tile([G, P], f32, tag="pmt")
                nc.tensor.transpose(pmt, Mtile, ident)
                nc.vector.tensor_copy(MT, pmt)
                nc.vector.reduce_max(dmax, MT, axis=AX.X)
                nc.gpsimd.indirect_dma_start(
                    out=minspRaw,
                    out_offset=None,
                    in_=spt_flat,
                    in_offset=IndirectOffsetOnAxis(ap=rowbase_i[:, :1], axis=0),
                )
                nc.vector.tensor_add(minspRaw, minspRaw, NMpre)
                nc.gpsimd.indirect_dma_start(
                    out=xsum,
                    out_offset=None,
                    in_=x_flat,
                    in_offset=IndirectOffsetOnAxis(ap=rowbase_i[:, :1], axis=0),
                )
                nc.gpsimd.dma_start(
                    at15,
                    attn[G - 1 : G]
                    .rearrange("g h (p t) j -> p g h t j", t=2)
                    .bitcast(f32r),
                )

        # ---- tail: scores -> bonus -> FPS iterations ----
        # masked scores straight out of PSUM (fused copy+mask), then smax
        scoresAll = cpool.tile([G, N], f32)
        nc.vector.tensor_mul(scoresAll, scores_ps, XM)
        smax = small.tile([G, 1], f32)
        nc.vector.reduce_max(smax, scoresAll, axis=AX.X)
        inv_smax = small.tile([G, 1], f32)
        nc.vector.reciprocal(inv_smax, smax)
        # bonusM = scores * (0.1 * dmax / smax) + NM  (dmax-scaled domain;
        # carries the -BIG marks of invalid + already-selected nodes)
        sfac = small.tile([G, 1], f32)
        nc.vector.tensor_scalar(
            sfac, inv_smax, dmax[:, :1], 0.1, op0=OP.mult, op1=OP.mult
        )
        bonusM = cpool.tile([G, N], f32)
        nc.vector.tensor_scalar(bonusM, scoresAll, sfac[:, :1], None, op0=OP.mult)

        # cand = candAcc (min-chain with bonus folded in; marks live in bonusM
        # and propagate through the min since spcol+bonusM >= -BIG there)
        candAcc = cpool.tile([G, N], f32)
        nc.vector.tensor_add(candAcc, minspRaw, bonusM)
        for t in range(1, K):
            mx8 = small.tile([G, 8], f32, tag="mx8")
            nc.vector.max(out=mx8, in_=candAcc)
            ix8 = small.tile([G, 8], u32, tag="ix8")
            nc.vector.max_index(ix8, mx8, candAcc)
            offi = small.tile([G, 1], i32, tag="offi")
            nc.vector.tensor_scalar(
                offi, ix8[:, 0:1], rowbase_f[:, :1], None, op0=OP.add
            )
            if t < K - 1:
                spcol = fps.tile([G, N], f32, tag="spcol")
                nc.gpsimd.indirect_dma_start(
                    out=spcol,
                    out_offset=None,
                    in_=spt_flat,
                    in_offset=IndirectOffsetOnAxis(ap=offi[:, :1], axis=0),
                )
            # accumulate the selected x row in-flight; the last row goes to
            # its own tile via a plain gather (a cce-add would serialize
            # behind the previous cce-add's completion, delaying the LN)
            if t < K - 1:
                nc.gpsimd.indirect_dma_start(
                    out=xsum,
                    out_offset=None,
                    in_=x_flat,
                    in_offset=IndirectOffsetOnAxis(ap=offi[:, :1], axis=0),
                    compute_op=OP.add,
                )
            else:
                x5 = cpool.tile([G, H], f32)
                nc.gpsimd.indirect_dma_start(
                    out=x5,
                    out_offset=None,
                    in_=x_flat,
                    in_offset=IndirectOffsetOnAxis(ap=offi[:, :1], axis=0),
                )
            if t < K - 1:
                # mark the selected node in bonusM (overlaps the gather)
                idxf = small.tile([G, 1], f32, tag="idxf")
                nc.vector.tensor_copy(idxf, ix8[:, 0:1])
                ohB = fps.tile([G, N], f32, tag="ohB")
                nc.vector.tensor_scalar(
                    ohB, iota_f, idxf[:, :1], -BIG, op0=OP.is_equal, op1=OP.mult
                )
                nc.vector.tensor_add(bonusM, bonusM, ohB)
                spadj = fps.tile([G, N], f32, tag="spadj")
                nc.vector.tensor_add(spadj, spcol, bonusM)
                nc.vector.tensor_tensor(candAcc, candAcc, spadj, op=OP.min)

        # ---- LayerNorm on xsum (mean of K rows; eps scaled by K^2) ----
        nc.vector.tensor_add(xsum, xsum, x5)
        st6 = small.tile([G, 6], f32)
        nc.vector.bn_stats(st6, xsum)
        mv = small.tile([G, 2], f32)
        nc.vector.bn_aggr(mv, st6)
        veps = small.tile([G, 1], f32)
        nc.vector.tensor_scalar(veps, mv[:, 1:2], float(K * K) * LN_EPS, None, op0=OP.add)
        std = small.tile([G, 1], f32)
        nc.scalar.sqrt(std, veps)
        rstd = small.tile([G, 1], f32)
        nc.vector.reciprocal(rstd, std)
        xn = cpool.tile([G, H], f32)
        nc.vector.tensor_scalar(
            xn, xsum, mv[:, 0:1], rstd[:, :1], op0=OP.subtract, op1=OP.mult
        )
        if trivial_affine:
            nc.sync.dma_start(out[:, :], xn)
        else:
            outt = cpool.tile([G, H], f32)
            nc.vector.tensor_mul(outt, xn, gb)
            nc.vector.tensor_add(outt, outt, bb)
            nc.sync.dma_start(out[:, :], outt)

    nc.compile()
    return nc


def core_inputs(core: int, x, attn, sp, xm, gamma, beta, trivial_affine=False) -> dict:
    """Per-core input map incl. host-precomputed constants."""
    sl = slice(core * G, (core + 1) * G)
    xmc = np.ascontiguousarray(xm[sl])  # [G, N]
    lens = xmc.sum(axis=1).astype(np.int32)  # [G]

    pidx = np.arange(P)
    # chunk masks: node p (c=0) / node 128+p (c=1) valid, interleaved pairs
    xmt2 = np.zeros((P, 2 * G), dtype=np.float32)
    xmt2[:, 0::2] = (pidx[:, None] < lens[None, :]).astype(np.float32)
    xmt2[:, 1::2] = ((pidx[:, None] + P) < lens[None, :]).astype(np.float32)
    # parity masks: node 2p / 2p+1 valid, in block-diagonal layout
    xme = (2 * pidx[:, None] < lens[None, :]).astype(np.float32)
    xmo = ((2 * pidx[:, None] + 1) < lens[None, :]).astype(np.float32)
    xbde = np.zeros((P, G * G), dtype=np.float32)
    xbdo = np.zeros((P, G * G), dtype=np.float32)
    for g in range(G):
        xbde[:, g * G + g] = xme[:, g]
        xbdo[:, g * G + g] = xmo[:, g]
    # NM: 0 on available nodes, -BIG on invalid + node 0
    nmpre = (xmc - 1.0) * BIG
    nmpre[:, 0] = -BIG
    iotaf = np.broadcast_to(
        np.arange(N, dtype=np.float32)[None, :], (G, N)
    ).copy()
    rowb = (np.arange(G, dtype=np.int64) * N).reshape(G, 1)
    ret_affine = {}
    if not trivial_affine:
        ret_affine = {
            "c_gb": np.broadcast_to(gamma.reshape(1, H), (G, H)).copy().astype(np.float32),
            "c_bb": np.broadcast_to(beta.reshape(1, H), (G, H)).copy().astype(np.float32),
        }
    return {
        **ret_affine,
        "x": np.ascontiguousarray(x[sl]),
        "attn": np.ascontiguousarray(attn[sl]),
        "spatial_pos": np.ascontiguousarray(sp[sl]),
        "x_mask": xmc,
        "c_xbde": xbde,
        "c_xbdo": xbdo,
        "c_xmt2": xmt2,
        "c_nmpre": nmpre.astype(np.float32),
        "c_iotaf": iotaf,
        "c_rowbi": rowb.astype(np.int32),
        "c_rowbf": rowb.astype(np.float32),
    }


_NC_CACHE = None


def kernel(**inputs) -> np.ndarray:
    global _NC_CACHE, LAST_RESULT
    from concourse.bass_utils import run_bass_kernel_spmd

    x = np.ascontiguousarray(np.asarray(inputs["x"]), dtype=np.float32)
    attn = np.ascontiguousarray(np.asarray(inputs["attn"]), dtype=np.float32)
    sp = np.ascontiguousarray(np.asarray(inputs["spatial_pos"]), dtype=np.float32)
    xm = np.ascontiguousarray(np.asarray(inputs["x_mask"]), dtype=np.float32)
    gamma = np.asarray(inputs["gamma"], dtype=np.float32)
    beta = np.asarray(inputs["beta"], dtype=np.float32)

    trivial = bool(np.all(gamma == 1.0) and np.all(beta == 0.0))
    if _NC_CACHE is None or _NC_CACHE[0] != trivial:
        _NC_CACHE = (trivial, build_bass(trivial_affine=trivial))
    nc = _NC_CACHE[1]

    in_maps = [
        core_inputs(c, x, attn, sp, xm, gamma, beta, trivial_affine=trivial)
        for c in range(NCORES)
    ]

    res = run_bass_kernel_spmd(
        nc, in_maps, core_ids=list(range(NCORES)), trace=TRACE
    )
    LAST_RESULT = res
    return np.concatenate([r["out"] for r in res.results], axis=0)
